# revision 1
# baseline (speedup 1.0000x reference)
"""Ensemble low-bit-decoded 3x3 conv2d, data-parallel over 8 TRN2 NeuronCores.

Problem (hardcoded): x (16, 64, 160, 160) f32. 4 ensemble members; image b uses
ensemble n = b % 4. Weights (64, 64, 3, 3) per ensemble are decoded on-device:
    w = scale_n * (sigmoid(clip(U_n*V_0)) + 2*sigmoid(clip(U_n*V_1)) - biasq_n - 4)
then out[b] = conv2d(x[b], w_{b%4}, pad=1) + bias_{b%4}.

Sharding: core j gets images (2j, 2j+1); decode params replicated (tiny).

Kernel strategy per image:
  SBUF "parity" layout: image rows (padded to 162 rows x 162 cols with zeros)
  stored as pairs: partition p<64 = channel ci of even padded row r'=2s,
  p>=64 = ci of odd r'=2s+1, at free column s*162 + col. A matmul with
  K=128 = (2 rows x 64 cin) and M=128 = (2 out rows x 64 cout) covers up to 4
  conv taps at once; 6 matmuls (2 row-phases x 3 kw shifts) accumulate a PSUM
  tile of 2-3 output row-pairs (F<=480), covering all 9 taps of the 3x3 stencil.
"""

import os

import numpy as np

import concourse.bass as bass
import concourse.mybir as mybir
import concourse.tile as tile
from concourse import bacc

N = 4
CIN = 64
COUT = 64
KS = 3
NB = 2  # weight bits
H = 160
W = 160
N_CORES = 8
N_IMG = 2  # images per core

F32 = mybir.dt.float32


def build_nc(
    n_img=N_IMG,
    h=H,
    w=W,
    band_out_pairs=20,
    st_pairs=3,
    mm_dtype=mybir.dt.float16,
):
    """Build the single-core Bass program (SPMD: all cores run this)."""
    wp = w + 2  # zero-padded width
    n_pairs = (h + 2) // 2  # padded row pairs in sbuf layout
    out_pairs = h // 2
    assert out_pairs % band_out_pairs == 0
    n_bands = out_pairs // band_out_pairs

    nc = bacc.Bacc("TRN2", target_bir_lowering=False, num_swdge_queues=4)

    x2 = nc.dram_tensor("x2", (n_img, CIN, h, w), F32, kind="ExternalInput")
    u2 = nc.dram_tensor("u2", (n_img, 128, 576), F32, kind="ExternalInput")
    v2 = nc.dram_tensor("v2", (NB, 128, 576), F32, kind="ExternalInput")
    wsc = nc.dram_tensor("wsc", (n_img, 128, 1), F32, kind="ExternalInput")
    woff = nc.dram_tensor("woff", (n_img, 128, 1), F32, kind="ExternalInput")
    bvec = nc.dram_tensor("bvec", (n_img, 128, 1), F32, kind="ExternalInput")
    out2 = nc.dram_tensor("out2", (n_img, COUT, h, w), F32, kind="ExternalOutput")

    AF = mybir.ActivationFunctionType
    OP = mybir.AluOpType

    with tile.TileContext(nc) as tc:
        with (
            tc.tile_pool(name="params", bufs=1) as ppool,
            tc.tile_pool(name="dec", bufs=2) as dpool,
            tc.tile_pool(name="wts", bufs=1) as wpool,
            tc.tile_pool(name="band", bufs=4) as bpool,
            tc.tile_pool(name="stage", bufs=4) as spool,
            tc.tile_pool(name="obuf", bufs=3) as opool,
            tc.tile_pool(name="psum", bufs=8, space="PSUM") as pspool,
        ):
            # spread bulk DMAs over the three DGE initiators (SP/ACT HWDGE
            # rings + gpsimd SWDGE) — each ring is its own FIFO to the SDMAs
            rings = [nc.sync, nc.scalar, nc.gpsimd]
            ring_state = [0]

            def next_ring():
                e = rings[ring_state[0] % len(rings)]
                ring_state[0] += 1
                return e
            # ---- shared V (both bit planes), stacked on 128 partitions
            v_sb = ppool.tile([128, NB, 576], F32, tag="v")
            nc.sync.dma_start(out=v_sb[:], in_=v2.rearrange("b p d -> p b d"))

            # ---- per-image decode of the 6 stacked lhsT weight tiles
            lhs = []  # lhs[i][widx] widx: 0..2 = phase1 kw, 3..5 = phase2 kw
            bias_sb = []
            for i in range(n_img):
                u_sb = dpool.tile([128, 576], F32, tag="u")
                nc.sync.dma_start(out=u_sb[:], in_=u2[i])
                wsc_sb = ppool.tile([128, 1], F32, tag=f"wsc{i}")
                woff_sb = ppool.tile([128, 1], F32, tag=f"woff{i}")
                bv_sb = ppool.tile([128, 1], F32, tag=f"bv{i}")
                nc.sync.dma_start(out=wsc_sb[:], in_=wsc[i])
                nc.sync.dma_start(out=woff_sb[:], in_=woff[i])
                nc.sync.dma_start(out=bv_sb[:], in_=bvec[i])
                bias_sb.append(bv_sb)

                s01 = []
                for b in range(NB):
                    t0 = dpool.tile([128, 576], F32, tag="t0")
                    nc.vector.tensor_mul(t0[:], u_sb[:], v_sb[:, b, :])
                    nc.vector.tensor_scalar(
                        t0[:], t0[:], 10.0, -10.0, op0=OP.min, op1=OP.max
                    )
                    s_b = dpool.tile([128, 576], F32, tag=f"s{b}")
                    nc.scalar.activation(s_b[:], t0[:], AF.Sigmoid)
                    s01.append(s_b)
                acc = dpool.tile([128, 576], F32, tag="acc")
                # acc = 2*s1 + s0
                nc.vector.scalar_tensor_tensor(
                    acc[:], s01[1][:], 2.0, s01[0][:], op0=OP.mult, op1=OP.add
                )
                wdec = dpool.tile([128, 576], F32, tag="wdec")
                # w = acc * scale + off   (off = -scale*(biasq+4))
                nc.scalar.activation(
                    wdec[:], acc[:], AF.Identity, bias=woff_sb[:], scale=wsc_sb[:]
                )
                w3 = wdec.rearrange("p (t c) -> p t c", t=9)  # t = kh*3+kw

                row = []
                for kw in range(KS):
                    l1 = wpool.tile([128, 2, 64], mm_dtype, tag=f"l1_{i}_{kw}")
                    l2 = wpool.tile([128, 2, 64], mm_dtype, tag=f"l2_{i}_{kw}")
                    nc.vector.memset(l1[:], 0.0)
                    nc.vector.memset(l2[:], 0.0)
                    # phase 1 (rhs rows 2m-1, 2m for out rows 2m, 2m+1):
                    #   (q0 -> j0): kh0   (q1 -> j0): kh1   (q1 -> j1): kh0
                    nc.vector.tensor_copy(l1[0:64, 0, :], w3[0:64, 0 * 3 + kw, :])
                    nc.vector.tensor_copy(l1[64:128, 0, :], w3[64:128, 1 * 3 + kw, :])
                    nc.vector.tensor_copy(l1[64:128, 1, :], w3[64:128, 0 * 3 + kw, :])
                    # phase 2 (rhs rows 2m+1, 2m+2):
                    #   (q0 -> j0): kh2   (q0 -> j1): kh1   (q1 -> j1): kh2
                    nc.vector.tensor_copy(l2[0:64, 0, :], w3[0:64, 2 * 3 + kw, :])
                    nc.vector.tensor_copy(l2[0:64, 1, :], w3[0:64, 1 * 3 + kw, :])
                    nc.vector.tensor_copy(l2[64:128, 1, :], w3[64:128, 2 * 3 + kw, :])
                    row.append((l1, l2))
                lhs.append([row[kw][0] for kw in range(KS)] + [row[kw][1] for kw in range(KS)])

            # ---- main conv loop
            # super-tile split of each band (out-pairs per PSUM tile)
            sts = []
            rem = band_out_pairs
            while rem > 0:
                k = min(st_pairs, rem)
                sts.append(k)
                rem -= k

            for i in range(n_img):
                for band in range(n_bands):
                    s0p = band * band_out_pairs  # first rhs pair == first out pair
                    s1p = s0p + band_out_pairs  # last rhs pair (inclusive)
                    npb = band_out_pairs + 1
                    # shared-pad layout: row-pair t's data at cols t*(w+1)+1..+w;
                    # col t*(w+1) is both row t's left pad and row t-1's right
                    # pad, so the matmul moving operand is 1D-contiguous.
                    wr = w + 1
                    bt = bpool.tile([128, npb * wr + 1], mm_dtype, tag="band")
                    b3 = bt[:, 0 : npb * wr].rearrange("p (t c) -> p t c", t=npb)
                    # zero pads (every wr-th col) + virtual edge rows
                    nc.vector.memset(bt[:, 0 : npb * wr + 1 : wr], 0.0)
                    if band == 0:
                        nc.vector.memset(b3[0:64, 0, 1 : w + 1], 0.0)
                    if band == n_bands - 1:
                        nc.vector.memset(b3[64:128, npb - 1, 1 : w + 1], 0.0)
                    # fp32 rows land in a staging tile via the two HWDGE rings
                    # + SWDGE (round-robin), then DVE casts into the fp16 band.
                    stg = spool.tile([128, npb, w], F32, tag="stage")
                    # q0 partitions (0:64) = odd real rows r=2s-1, s in [max(s0p,1), s1p]
                    a0 = max(s0p, 1)
                    cnt0 = s1p - a0 + 1
                    lo0 = a0 - s0p
                    next_ring().dma_start(
                        out=stg[0:64, lo0 : lo0 + cnt0, :],
                        in_=x2[i, :, 2 * a0 - 1 : 2 * s1p : 2, :],
                    )
                    # q1 partitions (64:128) = even real rows r=2s, s in [s0p, min(s1p, n_pairs-2)]
                    b1 = min(s1p, n_pairs - 2)
                    cnt1 = b1 - s0p + 1
                    next_ring().dma_start(
                        out=stg[64:128, 0:cnt1, :],
                        in_=x2[i, :, 2 * s0p : 2 * b1 + 1 : 2, :],
                    )
                    # cast fp32 -> fp16: common full-width region in one op,
                    # the single-parity edge rows separately
                    clo = max(lo0, 0)
                    chi = min(lo0 + cnt0, cnt1)
                    nc.vector.tensor_copy(
                        b3[:, clo:chi, 1 : w + 1], stg[:, clo:chi, :]
                    )
                    if clo > 0:  # band 0: q1-only row-pair 0
                        nc.vector.tensor_copy(
                            b3[64:128, 0:clo, 1 : w + 1], stg[64:128, 0:clo, :]
                        )
                    if lo0 + cnt0 > chi:  # last band: q0-only final pair
                        nc.vector.tensor_copy(
                            b3[0:64, chi : lo0 + cnt0, 1 : w + 1],
                            stg[0:64, chi : lo0 + cnt0, :],
                        )

                    psums = []
                    offs = []
                    o = 0
                    for k in sts:
                        psums.append(
                            pspool.tile([128, k * wr], F32, tag="ps", name="ps")
                        )
                        offs.append(o)
                        o += k

                    for widx in range(6):
                        kw = widx % 3
                        phase = widx // 3
                        lt = lhs[i][widx]
                        for sti, k in enumerate(sts):
                            base = (offs[sti] + phase) * wr
                            f = k * wr - 1
                            rhs = bt[:, base + kw : base + kw + f]
                            nc.tensor.matmul(
                                psums[sti][:, 0:f],
                                lt[:],
                                rhs,
                                start=(widx == 0),
                                stop=(widx == 5),
                            )

                    ob = opool.tile([128, band_out_pairs, w], F32, tag="ob")
                    for sti, k in enumerate(sts):
                        o = offs[sti]
                        ps3 = psums[sti].rearrange("p (t c) -> p t c", t=k)
                        nc.scalar.activation(
                            ob[:, o : o + k, :],
                            ps3[:, :, 0:w],
                            AF.Identity,
                            bias=bias_sb[i][:],
                            scale=1.0,
                        )
                    hh0 = 2 * s0p
                    hh1 = hh0 + 2 * band_out_pairs
                    next_ring().dma_start(out=out2[i, :, hh0:hh1:2, :], in_=ob[0:64])
                    next_ring().dma_start(
                        out=out2[i, :, hh0 + 1 : hh1 : 2, :], in_=ob[64:128]
                    )

    nc.compile()
    return nc


_NC_CACHE = {}


def _patch_ldw_opt():
    """Enable walrus LDWEIGHTS dedup: consecutive matmuls that reuse the same
    stationary operand skip the reload (bass_utils hardcodes it off)."""
    import concourse.bass_utils as bu

    if getattr(bu, "_ldw_patched", False):
        return
    orig = bu.run_command

    def patched(argv, **kwargs):
        argv = [
            "--enable-ldw-opt=true" if a == "--enable-ldw-opt=false" else a
            for a in argv
        ]
        return orig(argv, **kwargs)

    bu.run_command = patched
    bu._ldw_patched = True


def _get_nc():
    if "nc" not in _NC_CACHE:
        if os.environ.get("KERNEL_LDW_OPT"):
            # off by default: walrus codegen faults on deduped ldweights here
            _patch_ldw_opt()
        _NC_CACHE["nc"] = build_nc()
    return _NC_CACHE["nc"]


def _prep_params(U, V, scale, biasq, bias):
    """Host-side layout prep of the tiny decode parameters (per ensemble)."""
    # U (N, D, 1) with D laid out as (co, ci, kh, kw) -> (n, ci, kh*kw*co)
    up = U[:, :, 0].reshape(N, COUT, CIN, KS, KS).transpose(0, 2, 3, 4, 1)
    up = np.ascontiguousarray(up).reshape(N, CIN, 9 * COUT)
    ustack = np.concatenate([up, up], axis=1)  # (N, 128, 576)
    vp = V[:, :, 0].reshape(NB, COUT, CIN, KS, KS).transpose(0, 2, 3, 4, 1)
    vp = np.ascontiguousarray(vp).reshape(NB, CIN, 9 * COUT)
    vstack = np.concatenate([vp, vp], axis=1)  # (NB, 128, 576)
    sc = scale[:, 0]
    off = -sc * (biasq[:, 0] + 2.0**NB)
    wsc_n = np.tile(sc[:, None, None], (1, 128, 1)).astype(np.float32)
    woff_n = np.tile(off[:, None, None], (1, 128, 1)).astype(np.float32)
    bn = bias.reshape(N, COUT)
    bvec_n = np.concatenate([bn, bn], axis=1)[:, :, None].astype(np.float32)
    return (
        np.ascontiguousarray(ustack, np.float32),
        np.ascontiguousarray(vstack, np.float32),
        wsc_n,
        woff_n,
        bvec_n,
    )


LAST_RESULT = None


def _ensure_ntff_hook():
    """The container's antenv package lacks axon_hooks; synthesize it so
    run_bass_kernel_spmd(trace=True) can register the NTFF profiler."""
    import sys
    import types

    if "antenv.axon_hooks" in sys.modules:
        return True
    try:
        import antenv
        from trn_agent_boot.trn_boot import _ntff_profile_via_ctypes

        hook = _ntff_profile_via_ctypes("/opt/axon/libaxon_pjrt.so")
        mod = types.ModuleType("antenv.axon_hooks")
        mod._hook = hook
        mod.get_axon_ntff_profile_hook = lambda: mod._hook
        mod.set_axon_ntff_profile_hook = lambda h: setattr(mod, "_hook", h)
        sys.modules["antenv.axon_hooks"] = mod
        antenv.axon_hooks = mod
        return hook is not None
    except Exception as e:  # degrade to untraced run
        print(f"ntff hook setup failed: {type(e).__name__}: {e}")
        return False


def kernel(x, U, V, twopow, scale, biasq, bias):
    from concourse.bass_utils import run_bass_kernel_spmd

    global LAST_RESULT
    x = np.asarray(x, np.float32)
    ustack, vstack, wsc_n, woff_n, bvec_n = _prep_params(
        np.asarray(U, np.float32),
        np.asarray(V, np.float32),
        np.asarray(scale, np.float32),
        np.asarray(biasq, np.float32),
        np.asarray(bias, np.float32),
    )

    in_maps = []
    for j in range(N_CORES):
        bs = [N_IMG * j + t for t in range(N_IMG)]
        ns = [b % N for b in bs]
        in_maps.append(
            {
                "x2": np.ascontiguousarray(x[bs]),
                "u2": np.ascontiguousarray(ustack[ns]),
                "v2": vstack,
                "wsc": np.ascontiguousarray(wsc_n[ns]),
                "woff": np.ascontiguousarray(woff_n[ns]),
                "bvec": np.ascontiguousarray(bvec_n[ns]),
            }
        )

    nc = _get_nc()
    trace = bool(os.environ.get("KERNEL_TRACE"))
    if trace:
        trace = _ensure_ntff_hook()
    tmpdir = os.environ.get("KERNEL_TRACE_DIR") or None
    res = run_bass_kernel_spmd(
        nc, in_maps, list(range(N_CORES)), trace=trace, tmpdir=tmpdir
    )
    LAST_RESULT = res

    out = np.empty((16, COUT, H, W), np.float32)
    for j in range(N_CORES):
        out[N_IMG * j : N_IMG * (j + 1)] = res.results[j]["out2"]
    return out



# revision 3
# speedup vs baseline: 1.5884x; 1.5884x over previous
"""Ensemble low-bit-decoded 3x3 conv2d, data-parallel over 8 TRN2 NeuronCores.

Problem (hardcoded): x (16, 64, 160, 160) f32. 4 ensemble members; image b uses
ensemble n = b % 4. Weights (64, 64, 3, 3) per ensemble are decoded from tiny
U/V/scale/biasq params:
    w = scale_n * (sigmoid(clip(U_n*V_0)) + 2*sigmoid(clip(U_n*V_1)) - biasq_n - 4)
then out[b] = conv2d(x[b], w_{b%4}, pad=1) + bias_{b%4}.

Sharding: core j gets images (2j, 2j+1); weights decoded host-side (tiny) and
shipped as ready fp16 lhsT tiles.

Kernel strategy per image (pure conv on device, memory-roofline oriented):
  Host packs the image into a parity SBUF layout, fp16, zero pads baked in:
  partition ci<64 = channel ci of even padded row r'=2s, 64+ci = odd r'=2s+1,
  free offset s*161 + c (col 0 = shared left/right pad). All device DMAs are
  therefore large fully-contiguous-per-partition transfers. A matmul with
  K=128 = (2 rows x 64 cin), M=128 = (2 out rows x 64 cout) covers up to 4 conv
  taps; 6 matmuls (2 row-phases x 3 kw shifts) accumulate a PSUM supertile of
  2-3 output row-pairs (F<=482), covering all 9 taps of the 3x3 stencil.
  Output is written fp16 in the same parity layout and unpacked on host.
"""

import os

import numpy as np

import concourse.bass as bass
import concourse.mybir as mybir
import concourse.tile as tile
from concourse import bacc

N = 4
CIN = 64
COUT = 64
KS = 3
NB = 2  # weight bits
H = 160
W = 160
N_CORES = 8
N_IMG = 2  # images per core

PW = W + 1  # pair stride in the shared-pad layout
NPAIR = (H + 2) // 2  # 81 padded row-pairs
NPX = NPAIR * PW + 1  # free elements per image per partition (13042)
OUT_PAIRS = H // 2  # 80

F32 = mybir.dt.float32
F16 = mybir.dt.float16


def build_nc(n_img=N_IMG, band_out_pairs=20, st_pairs=3, n_in_chunks=8):
    """Build the single-core Bass program (SPMD: all cores run this)."""
    assert OUT_PAIRS % band_out_pairs == 0
    n_bands = OUT_PAIRS // band_out_pairs

    nc = bacc.Bacc("TRN2", target_bir_lowering=False, num_swdge_queues=4)

    x3 = nc.dram_tensor("x3", (n_img, 128, NPX), F16, kind="ExternalInput")
    wl = nc.dram_tensor("wl", (128, n_img * 6, 128), F16, kind="ExternalInput")
    bv = nc.dram_tensor("bv", (128, n_img), F32, kind="ExternalInput")
    out3 = nc.dram_tensor(
        "out3", (n_img, 128, OUT_PAIRS, W), F16, kind="ExternalOutput"
    )

    AF = mybir.ActivationFunctionType

    # supertile split of each band
    sts = []
    rem = band_out_pairs
    while rem > 0:
        k = min(st_pairs, rem)
        sts.append(k)
        rem -= k
    offs = []
    o = 0
    for k in sts:
        offs.append(o)
        o += k

    with tile.TileContext(nc) as tc:
        with (
            tc.tile_pool(name="wts", bufs=1) as wpool,
            tc.tile_pool(name="xbuf", bufs=1) as xpool,
            tc.tile_pool(name="obuf", bufs=4) as opool,
            tc.tile_pool(name="psum", bufs=8, space="PSUM") as pspool,
        ):
            w_sb = wpool.tile([128, n_img * 6, 128], F16, tag="w")
            nc.scalar.dma_start(out=w_sb[:], in_=wl[:, :, :])
            bv_sb = wpool.tile([128, n_img], F32, tag="bv")
            nc.scalar.dma_start(out=bv_sb[:], in_=bv[:, :])

            # whole input resident in SBUF, fp16 (26 KiB/partition/image);
            # chunked DMA so band 0's matmuls start early
            xt = xpool.tile([128, n_img, NPX], F16, tag="x")
            cpairs = NPAIR // n_in_chunks
            bnds = [cpairs * c * PW for c in range(n_in_chunks)] + [NPX]
            for i in range(n_img):
                for c in range(n_in_chunks):
                    nc.sync.dma_start(
                        out=xt[:, i, bnds[c] : bnds[c + 1]],
                        in_=x3[i, :, bnds[c] : bnds[c + 1]],
                    )

            for i in range(n_img):
                for band in range(n_bands):
                    s0 = band * band_out_pairs
                    ob = opool.tile([128, band_out_pairs, W], F16, tag="ob")
                    for sti, k in enumerate(sts):
                        ps = pspool.tile([128, k * PW], F32, tag="ps", name="ps")
                        f = k * PW - 1
                        for ph in range(2):
                            for kw in range(KS):
                                widx = ph * 3 + kw
                                a = (s0 + offs[sti] + ph) * PW + kw
                                nc.tensor.matmul(
                                    ps[:, 0:f],
                                    w_sb[:, i * 6 + widx, :],
                                    xt[:, i, a : a + f],
                                    start=(widx == 0),
                                    stop=(widx == 5),
                                )
                        ps3 = ps.rearrange("p (t c) -> p t c", t=k)
                        nc.scalar.activation(
                            ob[:, offs[sti] : offs[sti] + k, :],
                            ps3[:, :, 0:W],
                            AF.Identity,
                            bias=bv_sb[:, i : i + 1],
                            scale=1.0,
                        )
                    # two half-band output DMAs (SWDGE ring, decoupled from
                    # the scalar ACT stream) so the final tail is short
                    hb = band_out_pairs // 2
                    nc.gpsimd.dma_start(
                        out=out3[i, :, s0 : s0 + hb, :], in_=ob[:, 0:hb, :]
                    )
                    nc.gpsimd.dma_start(
                        out=out3[i, :, s0 + hb : s0 + band_out_pairs, :],
                        in_=ob[:, hb:band_out_pairs, :],
                    )

    nc.compile()
    return nc


_NC_CACHE = {}


def _get_nc():
    if "nc" not in _NC_CACHE:
        _NC_CACHE["nc"] = build_nc()
    return _NC_CACHE["nc"]


def _decode_weights(U, V, twopow, scale, biasq, bias):
    """Host-side decode of the tiny weight params -> fp16 lhsT tiles.

    Returns L (N, 6, 128, 128) fp16 and bn (N, 128) f32.
    lhsT tile widx = ph*3+kw, K index q = (row parity, cin), M = (out parity j,
    cout). Phase 1 reads rhs pair m (padded rows 2m, 2m+1), phase 2 pair m+1.
    """
    theta = np.einsum("ndk,bdk->nbd", U, V)  # (N, NB, D)
    sb = 1.0 / (1.0 + np.exp(-np.clip(theta, -10.0, 10.0)))
    integer = np.einsum("nbd,b->nd", sb, twopow)
    w = scale * (integer - biasq - 2.0**NB)  # (N, D)
    w = w.reshape(N, COUT, CIN, KS, KS)
    wq = np.ascontiguousarray(w.transpose(0, 2, 1, 3, 4))  # (n, ci, co, kh, kw)

    L = np.zeros((N, 6, 128, 128), np.float16)
    for kw in range(KS):
        # phase 1: (q0 -> j0): kh0, (q1 -> j0): kh1, (q1 -> j1): kh0
        L[:, kw, 0:64, 0:64] = wq[:, :, :, 0, kw]
        L[:, kw, 64:128, 0:64] = wq[:, :, :, 1, kw]
        L[:, kw, 64:128, 64:128] = wq[:, :, :, 0, kw]
        # phase 2: (q0 -> j0): kh2, (q0 -> j1): kh1, (q1 -> j1): kh2
        L[:, 3 + kw, 0:64, 0:64] = wq[:, :, :, 2, kw]
        L[:, 3 + kw, 0:64, 64:128] = wq[:, :, :, 1, kw]
        L[:, 3 + kw, 64:128, 64:128] = wq[:, :, :, 2, kw]

    bn = bias.reshape(N, COUT).astype(np.float32)
    bn = np.concatenate([bn, bn], axis=1)  # (N, 128)
    return L, bn


def _pack_x(xb):
    """(n, 64, 160, 160) f32 -> (n, 128, NPX) f16 parity layout, pads baked."""
    n = xb.shape[0]
    P = np.zeros((n, CIN, H + 2, H + 2), np.float16)
    P[:, :, 1 : H + 1, 1 : W + 1] = xb
    ev = P[:, :, 0 : H + 2 : 2, 0 : W + 1]  # (n, 64, 81, 161) padded rows 2s
    od = P[:, :, 1 : H + 2 : 2, 0 : W + 1]  # padded rows 2s+1
    arr = np.concatenate([ev, od], axis=1).reshape(n, 128, NPAIR * PW)
    out = np.zeros((n, 128, NPX), np.float16)
    out[:, :, 0 : NPAIR * PW] = arr
    return out


LAST_RESULT = None


def _ensure_ntff_hook():
    """The container's antenv package lacks axon_hooks; synthesize it so
    run_bass_kernel_spmd(trace=True) can register the NTFF profiler."""
    import sys
    import types

    if "antenv.axon_hooks" in sys.modules:
        return True
    try:
        import antenv
        from trn_agent_boot.trn_boot import _ntff_profile_via_ctypes

        hook = _ntff_profile_via_ctypes("/opt/axon/libaxon_pjrt.so")
        mod = types.ModuleType("antenv.axon_hooks")
        mod._hook = hook
        mod.get_axon_ntff_profile_hook = lambda: mod._hook
        mod.set_axon_ntff_profile_hook = lambda h: setattr(mod, "_hook", h)
        sys.modules["antenv.axon_hooks"] = mod
        antenv.axon_hooks = mod
        return hook is not None
    except Exception as e:  # degrade to untraced run
        print(f"ntff hook setup failed: {type(e).__name__}: {e}")
        return False


def kernel(x, U, V, twopow, scale, biasq, bias):
    from concourse.bass_utils import run_bass_kernel_spmd

    global LAST_RESULT
    x = np.asarray(x, np.float32)
    L, bn = _decode_weights(
        np.asarray(U, np.float32),
        np.asarray(V, np.float32),
        np.asarray(twopow, np.float32),
        np.asarray(scale, np.float32),
        np.asarray(biasq, np.float32),
        np.asarray(bias, np.float32),
    )

    in_maps = []
    for j in range(N_CORES):
        bs = [N_IMG * j + t for t in range(N_IMG)]
        ns = [b % N for b in bs]
        wlj = np.ascontiguousarray(
            L[ns].reshape(N_IMG * 6, 128, 128).transpose(1, 0, 2)
        )  # (128, n_img*6, 128)
        bvj = np.ascontiguousarray(bn[ns].T)  # (128, n_img)
        in_maps.append(
            {
                "x3": _pack_x(x[bs]),
                "wl": wlj,
                "bv": bvj,
            }
        )

    nc = _get_nc()
    trace = bool(os.environ.get("KERNEL_TRACE"))
    if trace:
        trace = _ensure_ntff_hook()
    tmpdir = os.environ.get("KERNEL_TRACE_DIR") or None
    res = run_bass_kernel_spmd(
        nc, in_maps, list(range(N_CORES)), trace=trace, tmpdir=tmpdir
    )
    LAST_RESULT = res

    out = np.empty((16, COUT, H, W), np.float32)
    for j in range(N_CORES):
        o3 = res.results[j]["out3"].astype(np.float32)  # (n_img, 128, 80, 160)
        for i in range(N_IMG):
            b = N_IMG * j + i
            out[b, :, 0::2, :] = o3[i, 0:64]
            out[b, :, 1::2, :] = o3[i, 64:128]
    return out


# revision 5
# speedup vs baseline: 1.6163x; 1.0176x over previous
"""Ensemble low-bit-decoded 3x3 conv2d, data-parallel over 8 TRN2 NeuronCores.

Problem (hardcoded): x (16, 64, 160, 160) f32. 4 ensemble members; image b uses
ensemble n = b % 4. Weights (64, 64, 3, 3) per ensemble are decoded from tiny
U/V/scale/biasq params:
    w = scale_n * (sigmoid(clip(U_n*V_0)) + 2*sigmoid(clip(U_n*V_1)) - biasq_n - 4)
then out[b] = conv2d(x[b], w_{b%4}, pad=1) + bias_{b%4}.

Sharding: core j gets images (2j, 2j+1); weights decoded host-side (tiny) and
shipped as ready fp16 lhsT tiles.

Kernel strategy per image (pure conv on device, memory-roofline oriented):
  Host packs the image into a parity SBUF layout, fp16, zero pads baked in:
  partition ci<64 = channel ci of even padded row r'=2s, 64+ci = odd r'=2s+1,
  free offset s*161 + c (col 0 = shared left/right pad). All device DMAs are
  therefore large fully-contiguous-per-partition transfers. A matmul with
  K=128 = (2 rows x 64 cin), M=128 = (2 out rows x 64 cout) covers up to 4 conv
  taps; 6 matmuls (2 row-phases x 3 kw shifts) accumulate a PSUM supertile of
  2-3 output row-pairs (F<=482), covering all 9 taps of the 3x3 stencil.
  Output is written fp16 in the same parity layout and unpacked on host.
"""

import os

import numpy as np

import concourse.bass as bass
import concourse.mybir as mybir
import concourse.tile as tile
from concourse import bacc

N = 4
CIN = 64
COUT = 64
KS = 3
NB = 2  # weight bits
H = 160
W = 160
N_CORES = 8
N_IMG = 2  # images per core

PW = W + 1  # pair stride in the shared-pad layout
NPAIR = (H + 2) // 2  # 81 padded row-pairs
NPX = NPAIR * PW + 1  # free elements per image per partition (13042)
OUT_PAIRS = H // 2  # 80

F32 = mybir.dt.float32
F16 = mybir.dt.float16


def build_nc(n_img=N_IMG, band_out_pairs=20, st_pairs=3, n_in_chunks=8):
    """Build the single-core Bass program (SPMD: all cores run this)."""
    assert OUT_PAIRS % band_out_pairs == 0
    n_bands = OUT_PAIRS // band_out_pairs

    nc = bacc.Bacc("TRN2", target_bir_lowering=False, num_swdge_queues=4)

    x3 = nc.dram_tensor("x3", (n_img, 128, NPX), F16, kind="ExternalInput")
    wl = nc.dram_tensor("wl", (128, n_img * 6, 128), F16, kind="ExternalInput")
    bv = nc.dram_tensor("bv", (128, n_img), F32, kind="ExternalInput")
    out3 = nc.dram_tensor(
        "out3", (n_img, 128, OUT_PAIRS, W), F16, kind="ExternalOutput"
    )

    AF = mybir.ActivationFunctionType

    # supertile split of each band
    sts = []
    rem = band_out_pairs
    while rem > 0:
        k = min(st_pairs, rem)
        sts.append(k)
        rem -= k
    offs = []
    o = 0
    for k in sts:
        offs.append(o)
        o += k

    with tile.TileContext(nc) as tc:
        with (
            tc.tile_pool(name="wts", bufs=1) as wpool,
            tc.tile_pool(name="xbuf", bufs=1) as xpool,
            tc.tile_pool(name="obuf", bufs=4) as opool,
            tc.tile_pool(name="psum", bufs=8, space="PSUM") as pspool,
        ):
            # HAM warmup: the PE clock sits at 1.2 GHz until ~3.4us of
            # sustained activity; burn that window on dummy matmuls over a
            # zeroed scratch tile while the first input chunks stream in.
            scr = wpool.tile([128, 512], F16, tag="scr")
            nc.vector.memset(scr[:], 0.0)
            for _ in range(6):
                wps = pspool.tile([128, 512], F32, tag="ps", name="ps")
                nc.tensor.matmul(
                    wps[:], scr[:, 0:128], scr[:], start=True, stop=True
                )

            w_sb = wpool.tile([128, n_img * 6, 128], F16, tag="w")
            nc.scalar.dma_start(out=w_sb[:], in_=wl[:, :, :])
            bv_sb = wpool.tile([128, n_img], F32, tag="bv")
            nc.scalar.dma_start(out=bv_sb[:], in_=bv[:, :])

            # whole input resident in SBUF, fp16 (26 KiB/partition/image);
            # chunked DMA so band 0's matmuls start early — image 0's first
            # band arrives in fine-grained (5-pair) chunks
            xt = xpool.tile([128, n_img, NPX], F16, tag="x")
            bnds0 = [0, 5 * PW, 10 * PW, 15 * PW, 21 * PW + 1] + [
                (31 + 10 * c) * PW + 1 for c in range(5)
            ] + [NPX]
            cpairs = NPAIR // n_in_chunks
            bnds1 = [cpairs * c * PW for c in range(n_in_chunks)] + [NPX]
            for i in range(n_img):
                bnds = bnds0 if i == 0 else bnds1
                for c in range(len(bnds) - 1):
                    nc.sync.dma_start(
                        out=xt[:, i, bnds[c] : bnds[c + 1]],
                        in_=x3[i, :, bnds[c] : bnds[c + 1]],
                    )

            for i in range(n_img):
                for band in range(n_bands):
                    s0 = band * band_out_pairs
                    last = i == n_img - 1 and band == n_bands - 1
                    ob = opool.tile([128, band_out_pairs, W], F16, tag="ob")
                    for sti, k in enumerate(sts):
                        ps = pspool.tile([128, k * PW], F32, tag="ps", name="ps")
                        f = k * PW - 1
                        for ph in range(2):
                            for kw in range(KS):
                                widx = ph * 3 + kw
                                a = (s0 + offs[sti] + ph) * PW + kw
                                nc.tensor.matmul(
                                    ps[:, 0:f],
                                    w_sb[:, i * 6 + widx, :],
                                    xt[:, i, a : a + f],
                                    start=(widx == 0),
                                    stop=(widx == 5),
                                )
                        ps3 = ps.rearrange("p (t c) -> p t c", t=k)
                        nc.scalar.activation(
                            ob[:, offs[sti] : offs[sti] + k, :],
                            ps3[:, :, 0:W],
                            AF.Identity,
                            bias=bv_sb[:, i : i + 1],
                            scale=1.0,
                        )
                    if not last:
                        # two half-band output DMAs (SWDGE ring, decoupled
                        # from the scalar ACT stream)
                        hb = band_out_pairs // 2
                        nc.gpsimd.dma_start(
                            out=out3[i, :, s0 : s0 + hb, :], in_=ob[:, 0:hb, :]
                        )
                        nc.gpsimd.dma_start(
                            out=out3[i, :, s0 + hb : s0 + band_out_pairs, :],
                            in_=ob[:, hb:band_out_pairs, :],
                        )
                    else:
                        # final band: four small chunks on the (now idle)
                        # sync HWDGE ring so the post-compute tail is short
                        qb = band_out_pairs // 4
                        for c in range(4):
                            nc.sync.dma_start(
                                out=out3[i, :, s0 + c * qb : s0 + (c + 1) * qb, :],
                                in_=ob[:, c * qb : (c + 1) * qb, :],
                            )

    nc.compile()
    return nc


_NC_CACHE = {}


def _get_nc():
    if "nc" not in _NC_CACHE:
        _NC_CACHE["nc"] = build_nc()
    return _NC_CACHE["nc"]


def _decode_weights(U, V, twopow, scale, biasq, bias):
    """Host-side decode of the tiny weight params -> fp16 lhsT tiles.

    Returns L (N, 6, 128, 128) fp16 and bn (N, 128) f32.
    lhsT tile widx = ph*3+kw, K index q = (row parity, cin), M = (out parity j,
    cout). Phase 1 reads rhs pair m (padded rows 2m, 2m+1), phase 2 pair m+1.
    """
    theta = np.einsum("ndk,bdk->nbd", U, V)  # (N, NB, D)
    sb = 1.0 / (1.0 + np.exp(-np.clip(theta, -10.0, 10.0)))
    integer = np.einsum("nbd,b->nd", sb, twopow)
    w = scale * (integer - biasq - 2.0**NB)  # (N, D)
    w = w.reshape(N, COUT, CIN, KS, KS)
    wq = np.ascontiguousarray(w.transpose(0, 2, 1, 3, 4))  # (n, ci, co, kh, kw)

    L = np.zeros((N, 6, 128, 128), np.float16)
    for kw in range(KS):
        # phase 1: (q0 -> j0): kh0, (q1 -> j0): kh1, (q1 -> j1): kh0
        L[:, kw, 0:64, 0:64] = wq[:, :, :, 0, kw]
        L[:, kw, 64:128, 0:64] = wq[:, :, :, 1, kw]
        L[:, kw, 64:128, 64:128] = wq[:, :, :, 0, kw]
        # phase 2: (q0 -> j0): kh2, (q0 -> j1): kh1, (q1 -> j1): kh2
        L[:, 3 + kw, 0:64, 0:64] = wq[:, :, :, 2, kw]
        L[:, 3 + kw, 0:64, 64:128] = wq[:, :, :, 1, kw]
        L[:, 3 + kw, 64:128, 64:128] = wq[:, :, :, 2, kw]

    bn = bias.reshape(N, COUT).astype(np.float32)
    bn = np.concatenate([bn, bn], axis=1)  # (N, 128)
    return L, bn


def _pack_x(xb):
    """(n, 64, 160, 160) f32 -> (n, 128, NPX) f16 parity layout, pads baked."""
    n = xb.shape[0]
    P = np.zeros((n, CIN, H + 2, H + 2), np.float16)
    P[:, :, 1 : H + 1, 1 : W + 1] = xb
    ev = P[:, :, 0 : H + 2 : 2, 0 : W + 1]  # (n, 64, 81, 161) padded rows 2s
    od = P[:, :, 1 : H + 2 : 2, 0 : W + 1]  # padded rows 2s+1
    arr = np.concatenate([ev, od], axis=1).reshape(n, 128, NPAIR * PW)
    out = np.zeros((n, 128, NPX), np.float16)
    out[:, :, 0 : NPAIR * PW] = arr
    return out


LAST_RESULT = None


def _ensure_ntff_hook():
    """The container's antenv package lacks axon_hooks; synthesize it so
    run_bass_kernel_spmd(trace=True) can register the NTFF profiler."""
    import sys
    import types

    if "antenv.axon_hooks" in sys.modules:
        return True
    try:
        import antenv
        from trn_agent_boot.trn_boot import _ntff_profile_via_ctypes

        hook = _ntff_profile_via_ctypes("/opt/axon/libaxon_pjrt.so")
        mod = types.ModuleType("antenv.axon_hooks")
        mod._hook = hook
        mod.get_axon_ntff_profile_hook = lambda: mod._hook
        mod.set_axon_ntff_profile_hook = lambda h: setattr(mod, "_hook", h)
        sys.modules["antenv.axon_hooks"] = mod
        antenv.axon_hooks = mod
        return hook is not None
    except Exception as e:  # degrade to untraced run
        print(f"ntff hook setup failed: {type(e).__name__}: {e}")
        return False


def kernel(x, U, V, twopow, scale, biasq, bias):
    from concourse.bass_utils import run_bass_kernel_spmd

    global LAST_RESULT
    x = np.asarray(x, np.float32)
    L, bn = _decode_weights(
        np.asarray(U, np.float32),
        np.asarray(V, np.float32),
        np.asarray(twopow, np.float32),
        np.asarray(scale, np.float32),
        np.asarray(biasq, np.float32),
        np.asarray(bias, np.float32),
    )

    in_maps = []
    for j in range(N_CORES):
        bs = [N_IMG * j + t for t in range(N_IMG)]
        ns = [b % N for b in bs]
        wlj = np.ascontiguousarray(
            L[ns].reshape(N_IMG * 6, 128, 128).transpose(1, 0, 2)
        )  # (128, n_img*6, 128)
        bvj = np.ascontiguousarray(bn[ns].T)  # (128, n_img)
        in_maps.append(
            {
                "x3": _pack_x(x[bs]),
                "wl": wlj,
                "bv": bvj,
            }
        )

    nc = _get_nc()
    trace = bool(os.environ.get("KERNEL_TRACE"))
    if trace:
        trace = _ensure_ntff_hook()
    tmpdir = os.environ.get("KERNEL_TRACE_DIR") or None
    res = run_bass_kernel_spmd(
        nc, in_maps, list(range(N_CORES)), trace=trace, tmpdir=tmpdir
    )
    LAST_RESULT = res

    out = np.empty((16, COUT, H, W), np.float32)
    for j in range(N_CORES):
        o3 = res.results[j]["out3"].astype(np.float32)  # (n_img, 128, 80, 160)
        for i in range(N_IMG):
            b = N_IMG * j + i
            out[b, :, 0::2, :] = o3[i, 0:64]
            out[b, :, 1::2, :] = o3[i, 64:128]
    return out


# revision 7
# speedup vs baseline: 1.6424x; 1.0161x over previous
"""Ensemble low-bit-decoded 3x3 conv2d, data-parallel over 8 TRN2 NeuronCores.

Problem (hardcoded): x (16, 64, 160, 160) f32. 4 ensemble members; image b uses
ensemble n = b % 4. Weights (64, 64, 3, 3) per ensemble are decoded from tiny
U/V/scale/biasq params:
    w = scale_n * (sigmoid(clip(U_n*V_0)) + 2*sigmoid(clip(U_n*V_1)) - biasq_n - 4)
then out[b] = conv2d(x[b], w_{b%4}, pad=1) + bias_{b%4}.

Sharding: core j gets images (2j, 2j+1); weights decoded host-side (tiny) and
shipped as ready fp16 lhsT tiles.

Kernel strategy per image (pure conv on device, memory-roofline oriented):
  Host packs the image into a parity SBUF layout, fp16, zero pads baked in:
  partition ci<64 = channel ci of even padded row r'=2s, 64+ci = odd r'=2s+1,
  free offset s*161 + c (col 0 = shared left/right pad). All device DMAs are
  therefore large fully-contiguous-per-partition transfers. A matmul with
  K=128 = (2 rows x 64 cin), M=128 = (2 out rows x 64 cout) covers up to 4 conv
  taps; 6 matmuls (2 row-phases x 3 kw shifts) accumulate a PSUM supertile of
  2-3 output row-pairs (F<=482), covering all 9 taps of the 3x3 stencil.
  Output is written fp16 in the same parity layout and unpacked on host.
"""

import os

import numpy as np

import concourse.bass as bass
import concourse.mybir as mybir
import concourse.tile as tile
from concourse import bacc

N = 4
CIN = 64
COUT = 64
KS = 3
NB = 2  # weight bits
H = 160
W = 160
N_CORES = 8
N_IMG = 2  # images per core

PW = W + 1  # pair stride in the shared-pad layout
NPAIR = (H + 2) // 2  # 81 padded row-pairs
NPX = NPAIR * PW + 1  # free elements per image per partition (13042)
OUT_PAIRS = H // 2  # 80

F32 = mybir.dt.float32
F16 = mybir.dt.float16


def build_nc(n_img=N_IMG, band_out_pairs=20, st_pairs=3, n_in_chunks=8):
    """Build the single-core Bass program (SPMD: all cores run this)."""
    assert OUT_PAIRS % band_out_pairs == 0
    n_bands = OUT_PAIRS // band_out_pairs

    nc = bacc.Bacc("TRN2", target_bir_lowering=False, num_swdge_queues=4)

    x3 = nc.dram_tensor("x3", (n_img, 128, NPX), F16, kind="ExternalInput")
    wl = nc.dram_tensor("wl", (128, n_img * 6, 128), F16, kind="ExternalInput")
    bv = nc.dram_tensor("bv", (128, n_img), F32, kind="ExternalInput")
    out3 = nc.dram_tensor(
        "out3", (n_img, 128, OUT_PAIRS, W), F16, kind="ExternalOutput"
    )

    AF = mybir.ActivationFunctionType

    # supertile split of each band
    sts = []
    rem = band_out_pairs
    while rem > 0:
        k = min(st_pairs, rem)
        sts.append(k)
        rem -= k
    offs = []
    o = 0
    for k in sts:
        offs.append(o)
        o += k

    with tile.TileContext(nc) as tc:
        with (
            tc.tile_pool(name="wts", bufs=1) as wpool,
            tc.tile_pool(name="xbuf", bufs=1) as xpool,
            tc.tile_pool(name="obuf", bufs=4) as opool,
            tc.tile_pool(name="psum", bufs=8, space="PSUM") as pspool,
        ):
            # HAM warmup: the PE clock sits at 1.2 GHz until ~3.4us of
            # sustained activity; burn that window on dummy matmuls over a
            # zeroed scratch tile while the first input chunks stream in.
            scr = wpool.tile([128, 512], F16, tag="scr")
            nc.vector.memset(scr[:], 0.0)
            for _ in range(7):
                wps = pspool.tile([128, 512], F32, tag="ps", name="ps")
                nc.tensor.matmul(
                    wps[:], scr[:, 0:128], scr[:], start=True, stop=True
                )

            # weights first on the sync ring: they gate the first real matmul
            w_sb = wpool.tile([128, n_img * 6, 128], F16, tag="w")
            nc.sync.dma_start(out=w_sb[:], in_=wl[:, :, :])
            bv_sb = wpool.tile([128, n_img], F32, tag="bv")
            nc.scalar.dma_start(out=bv_sb[:], in_=bv[:, :])

            # whole input resident in SBUF, fp16 (26 KiB/partition/image);
            # chunked DMA so band 0's matmuls start early — image 0's first
            # band arrives in fine-grained (5-pair) chunks
            xt = xpool.tile([128, n_img, NPX], F16, tag="x")
            bnds0 = [0, 5 * PW, 10 * PW, 15 * PW, 21 * PW + 1] + [
                (31 + 10 * c) * PW + 1 for c in range(5)
            ] + [NPX]
            cpairs = NPAIR // n_in_chunks
            bnds1 = [cpairs * c * PW for c in range(n_in_chunks)] + [NPX]
            for i in range(n_img):
                bnds = bnds0 if i == 0 else bnds1
                for c in range(len(bnds) - 1):
                    nc.sync.dma_start(
                        out=xt[:, i, bnds[c] : bnds[c + 1]],
                        in_=x3[i, :, bnds[c] : bnds[c + 1]],
                    )

            for i in range(n_img):
                for band in range(n_bands):
                    s0 = band * band_out_pairs
                    last = i == n_img - 1 and band == n_bands - 1
                    ob = opool.tile([128, band_out_pairs, W], F16, tag="ob")
                    # final band streams out in shrinking chunks on the (by
                    # then idle) sync HWDGE ring so the post-compute tail is
                    # just a 2-pair store; (lo, hi, after-supertile)
                    ochunks = [(0, 9, 3), (9, 15, 4), (15, 18, 5), (18, 20, 6)]
                    for sti, k in enumerate(sts):
                        ps = pspool.tile([128, k * PW], F32, tag="ps", name="ps")
                        f = k * PW - 1
                        for ph in range(2):
                            for kw in range(KS):
                                widx = ph * 3 + kw
                                a = (s0 + offs[sti] + ph) * PW + kw
                                nc.tensor.matmul(
                                    ps[:, 0:f],
                                    w_sb[:, i * 6 + widx, :],
                                    xt[:, i, a : a + f],
                                    start=(widx == 0),
                                    stop=(widx == 5),
                                )
                        ps3 = ps.rearrange("p (t c) -> p t c", t=k)
                        nc.scalar.activation(
                            ob[:, offs[sti] : offs[sti] + k, :],
                            ps3[:, :, 0:W],
                            AF.Identity,
                            bias=bv_sb[:, i : i + 1],
                            scale=1.0,
                        )
                        if last:
                            for lo, hi, after in ochunks:
                                if after == sti:
                                    nc.sync.dma_start(
                                        out=out3[i, :, s0 + lo : s0 + hi, :],
                                        in_=ob[:, lo:hi, :],
                                    )
                    if not last:
                        # two half-band output DMAs (SWDGE ring, decoupled
                        # from the scalar ACT stream)
                        hb = band_out_pairs // 2
                        nc.gpsimd.dma_start(
                            out=out3[i, :, s0 : s0 + hb, :], in_=ob[:, 0:hb, :]
                        )
                        nc.gpsimd.dma_start(
                            out=out3[i, :, s0 + hb : s0 + band_out_pairs, :],
                            in_=ob[:, hb:band_out_pairs, :],
                        )

    nc.compile()
    return nc


_NC_CACHE = {}


def _get_nc():
    if "nc" not in _NC_CACHE:
        _NC_CACHE["nc"] = build_nc()
    return _NC_CACHE["nc"]


def _decode_weights(U, V, twopow, scale, biasq, bias):
    """Host-side decode of the tiny weight params -> fp16 lhsT tiles.

    Returns L (N, 6, 128, 128) fp16 and bn (N, 128) f32.
    lhsT tile widx = ph*3+kw, K index q = (row parity, cin), M = (out parity j,
    cout). Phase 1 reads rhs pair m (padded rows 2m, 2m+1), phase 2 pair m+1.
    """
    theta = np.einsum("ndk,bdk->nbd", U, V)  # (N, NB, D)
    sb = 1.0 / (1.0 + np.exp(-np.clip(theta, -10.0, 10.0)))
    integer = np.einsum("nbd,b->nd", sb, twopow)
    w = scale * (integer - biasq - 2.0**NB)  # (N, D)
    w = w.reshape(N, COUT, CIN, KS, KS)
    wq = np.ascontiguousarray(w.transpose(0, 2, 1, 3, 4))  # (n, ci, co, kh, kw)

    L = np.zeros((N, 6, 128, 128), np.float16)
    for kw in range(KS):
        # phase 1: (q0 -> j0): kh0, (q1 -> j0): kh1, (q1 -> j1): kh0
        L[:, kw, 0:64, 0:64] = wq[:, :, :, 0, kw]
        L[:, kw, 64:128, 0:64] = wq[:, :, :, 1, kw]
        L[:, kw, 64:128, 64:128] = wq[:, :, :, 0, kw]
        # phase 2: (q0 -> j0): kh2, (q0 -> j1): kh1, (q1 -> j1): kh2
        L[:, 3 + kw, 0:64, 0:64] = wq[:, :, :, 2, kw]
        L[:, 3 + kw, 0:64, 64:128] = wq[:, :, :, 1, kw]
        L[:, 3 + kw, 64:128, 64:128] = wq[:, :, :, 2, kw]

    bn = bias.reshape(N, COUT).astype(np.float32)
    bn = np.concatenate([bn, bn], axis=1)  # (N, 128)
    return L, bn


def _pack_x(xb):
    """(n, 64, 160, 160) f32 -> (n, 128, NPX) f16 parity layout, pads baked."""
    n = xb.shape[0]
    P = np.zeros((n, CIN, H + 2, H + 2), np.float16)
    P[:, :, 1 : H + 1, 1 : W + 1] = xb
    ev = P[:, :, 0 : H + 2 : 2, 0 : W + 1]  # (n, 64, 81, 161) padded rows 2s
    od = P[:, :, 1 : H + 2 : 2, 0 : W + 1]  # padded rows 2s+1
    arr = np.concatenate([ev, od], axis=1).reshape(n, 128, NPAIR * PW)
    out = np.zeros((n, 128, NPX), np.float16)
    out[:, :, 0 : NPAIR * PW] = arr
    return out


LAST_RESULT = None


def _ensure_ntff_hook():
    """The container's antenv package lacks axon_hooks; synthesize it so
    run_bass_kernel_spmd(trace=True) can register the NTFF profiler."""
    import sys
    import types

    if "antenv.axon_hooks" in sys.modules:
        return True
    try:
        import antenv
        from trn_agent_boot.trn_boot import _ntff_profile_via_ctypes

        hook = _ntff_profile_via_ctypes("/opt/axon/libaxon_pjrt.so")
        mod = types.ModuleType("antenv.axon_hooks")
        mod._hook = hook
        mod.get_axon_ntff_profile_hook = lambda: mod._hook
        mod.set_axon_ntff_profile_hook = lambda h: setattr(mod, "_hook", h)
        sys.modules["antenv.axon_hooks"] = mod
        antenv.axon_hooks = mod
        return hook is not None
    except Exception as e:  # degrade to untraced run
        print(f"ntff hook setup failed: {type(e).__name__}: {e}")
        return False


def kernel(x, U, V, twopow, scale, biasq, bias):
    from concourse.bass_utils import run_bass_kernel_spmd

    global LAST_RESULT
    x = np.asarray(x, np.float32)
    L, bn = _decode_weights(
        np.asarray(U, np.float32),
        np.asarray(V, np.float32),
        np.asarray(twopow, np.float32),
        np.asarray(scale, np.float32),
        np.asarray(biasq, np.float32),
        np.asarray(bias, np.float32),
    )

    in_maps = []
    for j in range(N_CORES):
        bs = [N_IMG * j + t for t in range(N_IMG)]
        ns = [b % N for b in bs]
        wlj = np.ascontiguousarray(
            L[ns].reshape(N_IMG * 6, 128, 128).transpose(1, 0, 2)
        )  # (128, n_img*6, 128)
        bvj = np.ascontiguousarray(bn[ns].T)  # (128, n_img)
        in_maps.append(
            {
                "x3": _pack_x(x[bs]),
                "wl": wlj,
                "bv": bvj,
            }
        )

    nc = _get_nc()
    trace = bool(os.environ.get("KERNEL_TRACE"))
    if trace:
        trace = _ensure_ntff_hook()
    tmpdir = os.environ.get("KERNEL_TRACE_DIR") or None
    res = run_bass_kernel_spmd(
        nc, in_maps, list(range(N_CORES)), trace=trace, tmpdir=tmpdir
    )
    LAST_RESULT = res

    out = np.empty((16, COUT, H, W), np.float32)
    for j in range(N_CORES):
        o3 = res.results[j]["out3"].astype(np.float32)  # (n_img, 128, 80, 160)
        for i in range(N_IMG):
            b = N_IMG * j + i
            out[b, :, 0::2, :] = o3[i, 0:64]
            out[b, :, 1::2, :] = o3[i, 64:128]
    return out


# revision 8
# speedup vs baseline: 1.6430x; 1.0004x over previous
"""Ensemble low-bit-decoded 3x3 conv2d, data-parallel over 8 TRN2 NeuronCores.

Problem (hardcoded): x (16, 64, 160, 160) f32. 4 ensemble members; image b uses
ensemble n = b % 4. Weights (64, 64, 3, 3) per ensemble are decoded from tiny
U/V/scale/biasq params:
    w = scale_n * (sigmoid(clip(U_n*V_0)) + 2*sigmoid(clip(U_n*V_1)) - biasq_n - 4)
then out[b] = conv2d(x[b], w_{b%4}, pad=1) + bias_{b%4}.

Sharding: core j gets images (2j, 2j+1); weights decoded host-side (tiny) and
shipped as ready fp16 lhsT tiles.

Kernel strategy per image (pure conv on device, memory-roofline oriented):
  Host packs the image into a parity SBUF layout, fp16, zero pads baked in:
  partition ci<64 = channel ci of even padded row r'=2s, 64+ci = odd r'=2s+1,
  free offset s*161 + c (col 0 = shared left/right pad). All device DMAs are
  therefore large fully-contiguous-per-partition transfers. A matmul with
  K=128 = (2 rows x 64 cin), M=128 = (2 out rows x 64 cout) covers up to 4 conv
  taps; 6 matmuls (2 row-phases x 3 kw shifts) accumulate a PSUM supertile of
  2-3 output row-pairs (F<=482), covering all 9 taps of the 3x3 stencil.
  Output is written fp16 in the same parity layout and unpacked on host.
"""

import os

import numpy as np

import concourse.bass as bass
import concourse.mybir as mybir
import concourse.tile as tile
from concourse import bacc

N = 4
CIN = 64
COUT = 64
KS = 3
NB = 2  # weight bits
H = 160
W = 160
N_CORES = 8
N_IMG = 2  # images per core

PW = W + 1  # pair stride in the shared-pad layout
NPAIR = (H + 2) // 2  # 81 padded row-pairs
NPX = NPAIR * PW + 1  # free elements per image per partition (13042)
OUT_PAIRS = H // 2  # 80

F32 = mybir.dt.float32
F16 = mybir.dt.float16


def build_nc(n_img=N_IMG, band_out_pairs=20, st_pairs=3, n_in_chunks=8):
    """Build the single-core Bass program (SPMD: all cores run this)."""
    assert OUT_PAIRS % band_out_pairs == 0
    n_bands = OUT_PAIRS // band_out_pairs

    nc = bacc.Bacc("TRN2", target_bir_lowering=False, num_swdge_queues=4)

    x3 = nc.dram_tensor("x3", (n_img, 128, NPX), F16, kind="ExternalInput")
    wl = nc.dram_tensor("wl", (128, n_img * 6, 128), F16, kind="ExternalInput")
    bv = nc.dram_tensor("bv", (128, n_img), F32, kind="ExternalInput")
    out3 = nc.dram_tensor(
        "out3", (n_img, 128, OUT_PAIRS, W), F16, kind="ExternalOutput"
    )

    AF = mybir.ActivationFunctionType

    # supertile split of each band
    sts = []
    rem = band_out_pairs
    while rem > 0:
        k = min(st_pairs, rem)
        sts.append(k)
        rem -= k
    offs = []
    o = 0
    for k in sts:
        offs.append(o)
        o += k

    with tile.TileContext(nc) as tc:
        with (
            tc.tile_pool(name="wts", bufs=1) as wpool,
            tc.tile_pool(name="xbuf", bufs=1) as xpool,
            tc.tile_pool(name="obuf", bufs=4) as opool,
            tc.tile_pool(name="psum", bufs=8, space="PSUM") as pspool,
        ):
            # HAM warmup: the PE clock sits at 1.2 GHz until ~3.4us of
            # sustained activity; burn that window on dummy matmuls over a
            # zeroed scratch tile while the first input chunks stream in.
            scr = wpool.tile([128, 512], F16, tag="scr")
            nc.vector.memset(scr[:], 0.0)
            for _ in range(5):
                wps = pspool.tile([128, 512], F32, tag="ps", name="ps")
                nc.tensor.matmul(
                    wps[:], scr[:, 0:128], scr[:], start=True, stop=True
                )

            # startup-critical loads split across BOTH HWDGE rings (plus the
            # SWDGE ring for later chunks) so the cold-HBM descriptor latency
            # overlaps: image-0 weights on sync || first x chunk on scalar
            w_sb = wpool.tile([128, n_img * 6, 128], F16, tag="w")
            bv_sb = wpool.tile([128, n_img], F32, tag="bv")
            xt = xpool.tile([128, n_img, NPX], F16, tag="x")

            bnds0 = [0, 5 * PW, 10 * PW, 15 * PW, 21 * PW + 1] + [
                (31 + 10 * c) * PW + 1 for c in range(5)
            ] + [NPX]
            x0c = [(0, bnds0[c], bnds0[c + 1]) for c in range(len(bnds0) - 1)]
            cpairs = NPAIR // n_in_chunks
            bnds1 = [cpairs * c * PW for c in range(n_in_chunks)] + [NPX]
            x1c = [(1, bnds1[c], bnds1[c + 1]) for c in range(len(bnds1) - 1)]

            def xdma(eng, i, lo, hi):
                eng.dma_start(
                    out=xt[:, i, lo:hi], in_=x3[i, :, lo:hi]
                )

            # sync ring: image-0 weights, then odd image-0 chunks + image-1
            nc.sync.dma_start(out=w_sb[:, 0:6, :], in_=wl[:, 0:6, :])
            for i, lo, hi in [x0c[1], x0c[3], x0c[5], x0c[7], x0c[9],
                              x1c[0], x1c[2], x1c[4], x1c[6]]:
                xdma(nc.sync, i, lo, hi)
            # scalar ring: first x chunk (gates the first matmul), a couple
            # more, then bias + image-1 weights
            xdma(nc.scalar, *x0c[0])
            xdma(nc.scalar, *x0c[2])
            nc.scalar.dma_start(out=bv_sb[:], in_=bv[:, :])
            nc.scalar.dma_start(out=w_sb[:, 6:12, :], in_=wl[:, 6:12, :])
            # gpsimd (SWDGE) ring: mid chunks, idle otherwise at this point
            for i, lo, hi in [x0c[4], x0c[6], x0c[8],
                              x1c[1], x1c[3], x1c[5], x1c[7]]:
                xdma(nc.gpsimd, i, lo, hi)

            for i in range(n_img):
                for band in range(n_bands):
                    s0 = band * band_out_pairs
                    last = i == n_img - 1 and band == n_bands - 1
                    ob = opool.tile([128, band_out_pairs, W], F16, tag="ob")
                    # final band streams out in shrinking chunks on the (by
                    # then idle) sync HWDGE ring so the post-compute tail is
                    # just a 2-pair store; (lo, hi, after-supertile)
                    ochunks = [(0, 9, 3), (9, 15, 4), (15, 18, 5), (18, 20, 6)]
                    for sti, k in enumerate(sts):
                        ps = pspool.tile([128, k * PW], F32, tag="ps", name="ps")
                        f = k * PW - 1
                        for ph in range(2):
                            for kw in range(KS):
                                widx = ph * 3 + kw
                                a = (s0 + offs[sti] + ph) * PW + kw
                                nc.tensor.matmul(
                                    ps[:, 0:f],
                                    w_sb[:, i * 6 + widx, :],
                                    xt[:, i, a : a + f],
                                    start=(widx == 0),
                                    stop=(widx == 5),
                                )
                        ps3 = ps.rearrange("p (t c) -> p t c", t=k)
                        nc.scalar.activation(
                            ob[:, offs[sti] : offs[sti] + k, :],
                            ps3[:, :, 0:W],
                            AF.Identity,
                            bias=bv_sb[:, i : i + 1],
                            scale=1.0,
                        )
                        if last:
                            for lo, hi, after in ochunks:
                                if after == sti:
                                    nc.sync.dma_start(
                                        out=out3[i, :, s0 + lo : s0 + hi, :],
                                        in_=ob[:, lo:hi, :],
                                    )
                    if not last:
                        # two half-band output DMAs (SWDGE ring, decoupled
                        # from the scalar ACT stream)
                        hb = band_out_pairs // 2
                        nc.gpsimd.dma_start(
                            out=out3[i, :, s0 : s0 + hb, :], in_=ob[:, 0:hb, :]
                        )
                        nc.gpsimd.dma_start(
                            out=out3[i, :, s0 + hb : s0 + band_out_pairs, :],
                            in_=ob[:, hb:band_out_pairs, :],
                        )

    nc.compile()
    return nc


_NC_CACHE = {}


def _get_nc():
    if "nc" not in _NC_CACHE:
        _NC_CACHE["nc"] = build_nc()
    return _NC_CACHE["nc"]


def _decode_weights(U, V, twopow, scale, biasq, bias):
    """Host-side decode of the tiny weight params -> fp16 lhsT tiles.

    Returns L (N, 6, 128, 128) fp16 and bn (N, 128) f32.
    lhsT tile widx = ph*3+kw, K index q = (row parity, cin), M = (out parity j,
    cout). Phase 1 reads rhs pair m (padded rows 2m, 2m+1), phase 2 pair m+1.
    """
    theta = np.einsum("ndk,bdk->nbd", U, V)  # (N, NB, D)
    sb = 1.0 / (1.0 + np.exp(-np.clip(theta, -10.0, 10.0)))
    integer = np.einsum("nbd,b->nd", sb, twopow)
    w = scale * (integer - biasq - 2.0**NB)  # (N, D)
    w = w.reshape(N, COUT, CIN, KS, KS)
    wq = np.ascontiguousarray(w.transpose(0, 2, 1, 3, 4))  # (n, ci, co, kh, kw)

    L = np.zeros((N, 6, 128, 128), np.float16)
    for kw in range(KS):
        # phase 1: (q0 -> j0): kh0, (q1 -> j0): kh1, (q1 -> j1): kh0
        L[:, kw, 0:64, 0:64] = wq[:, :, :, 0, kw]
        L[:, kw, 64:128, 0:64] = wq[:, :, :, 1, kw]
        L[:, kw, 64:128, 64:128] = wq[:, :, :, 0, kw]
        # phase 2: (q0 -> j0): kh2, (q0 -> j1): kh1, (q1 -> j1): kh2
        L[:, 3 + kw, 0:64, 0:64] = wq[:, :, :, 2, kw]
        L[:, 3 + kw, 0:64, 64:128] = wq[:, :, :, 1, kw]
        L[:, 3 + kw, 64:128, 64:128] = wq[:, :, :, 2, kw]

    bn = bias.reshape(N, COUT).astype(np.float32)
    bn = np.concatenate([bn, bn], axis=1)  # (N, 128)
    return L, bn


def _pack_x(xb):
    """(n, 64, 160, 160) f32 -> (n, 128, NPX) f16 parity layout, pads baked."""
    n = xb.shape[0]
    P = np.zeros((n, CIN, H + 2, H + 2), np.float16)
    P[:, :, 1 : H + 1, 1 : W + 1] = xb
    ev = P[:, :, 0 : H + 2 : 2, 0 : W + 1]  # (n, 64, 81, 161) padded rows 2s
    od = P[:, :, 1 : H + 2 : 2, 0 : W + 1]  # padded rows 2s+1
    arr = np.concatenate([ev, od], axis=1).reshape(n, 128, NPAIR * PW)
    out = np.zeros((n, 128, NPX), np.float16)
    out[:, :, 0 : NPAIR * PW] = arr
    return out


LAST_RESULT = None


def _ensure_ntff_hook():
    """The container's antenv package lacks axon_hooks; synthesize it so
    run_bass_kernel_spmd(trace=True) can register the NTFF profiler."""
    import sys
    import types

    if "antenv.axon_hooks" in sys.modules:
        return True
    try:
        import antenv
        from trn_agent_boot.trn_boot import _ntff_profile_via_ctypes

        hook = _ntff_profile_via_ctypes("/opt/axon/libaxon_pjrt.so")
        mod = types.ModuleType("antenv.axon_hooks")
        mod._hook = hook
        mod.get_axon_ntff_profile_hook = lambda: mod._hook
        mod.set_axon_ntff_profile_hook = lambda h: setattr(mod, "_hook", h)
        sys.modules["antenv.axon_hooks"] = mod
        antenv.axon_hooks = mod
        return hook is not None
    except Exception as e:  # degrade to untraced run
        print(f"ntff hook setup failed: {type(e).__name__}: {e}")
        return False


def kernel(x, U, V, twopow, scale, biasq, bias):
    from concourse.bass_utils import run_bass_kernel_spmd

    global LAST_RESULT
    x = np.asarray(x, np.float32)
    L, bn = _decode_weights(
        np.asarray(U, np.float32),
        np.asarray(V, np.float32),
        np.asarray(twopow, np.float32),
        np.asarray(scale, np.float32),
        np.asarray(biasq, np.float32),
        np.asarray(bias, np.float32),
    )

    in_maps = []
    for j in range(N_CORES):
        bs = [N_IMG * j + t for t in range(N_IMG)]
        ns = [b % N for b in bs]
        wlj = np.ascontiguousarray(
            L[ns].reshape(N_IMG * 6, 128, 128).transpose(1, 0, 2)
        )  # (128, n_img*6, 128)
        bvj = np.ascontiguousarray(bn[ns].T)  # (128, n_img)
        in_maps.append(
            {
                "x3": _pack_x(x[bs]),
                "wl": wlj,
                "bv": bvj,
            }
        )

    nc = _get_nc()
    trace = bool(os.environ.get("KERNEL_TRACE"))
    if trace:
        trace = _ensure_ntff_hook()
    tmpdir = os.environ.get("KERNEL_TRACE_DIR") or None
    res = run_bass_kernel_spmd(
        nc, in_maps, list(range(N_CORES)), trace=trace, tmpdir=tmpdir
    )
    LAST_RESULT = res

    out = np.empty((16, COUT, H, W), np.float32)
    for j in range(N_CORES):
        o3 = res.results[j]["out3"].astype(np.float32)  # (n_img, 128, 80, 160)
        for i in range(N_IMG):
            b = N_IMG * j + i
            out[b, :, 0::2, :] = o3[i, 0:64]
            out[b, :, 1::2, :] = o3[i, 64:128]
    return out
